# revision 1
# baseline (speedup 1.0000x reference)
"""Adaptive average pool 2D (64x64 -> 6x6) on 8 TRN2 NeuronCores.

Input  x: (16, 2048, 64, 64) f32
Output  : (16, 2048, 6, 6) f32

Sharding: data-parallel over the batch dim (2 batches per core).
Per-core kernel: 32 tiles of [128 channels (partitions), 4096 (h*w)].
Stage 1 reduces W (64 -> 6 bins), stage 2 reduces H (two strided DVE
reduces per stage — bin starts decompose as {10,42} and {0,32}x{0,21}),
then the 1/(bin_h*bin_w) mean scale runs on the ACT engine.
"""

import numpy as np

import concourse.bass as bass
import concourse.bacc as bacc
import concourse.mybir as mybir
from concourse.ap import AP
from concourse.tile import TileContext
from concourse.bass_utils import run_bass_kernel_spmd

# ---------------------------------------------------------------- problem dims
N, C, H_IN, W_IN = 16, 2048, 64, 64
H_OUT = W_OUT = 6
N_CORES = 8
N_PER_CORE = N // N_CORES          # 2
CBLK = 128
ROWS = N_PER_CORE * C              # 4096 (n,c) rows per core
NTILES = ROWS // CBLK              # 32
HW = H_IN * W_IN                   # 4096
F32 = mybir.dt.float32

# Adaptive pool bin edges (floor/ceil rule, same for H and W since 64->6).
# Bin sizes are [11,12,11,11,12,11] at starts [0,10,21,32,42,53].  The two
# 12-size bins (1,4) start at {10,42} (stride 32); the four 11-size bins
# (0,2,3,5) start at {0,21,32,53} = {0,32} x {0,21}, a 2-D stride
# decomposition — so each pooling stage is just TWO strided DVE reduces.
_STARTS = [0, 10, 21, 32, 42, 53]
_SIZES = [11, 12, 11, 11, 12, 11]
# Scale groups for the mean division: (bin-index pair, size).
_SGROUPS = [((1, 4), 12), ((0, 3), 11), ((2, 5), 11)]


def _ap(t: bass.AP, off: int, dims) -> AP:
    """Custom strided AP on a pool tile (keeps the tile's partition dim)."""
    assert t.offset == 0
    return AP(t.tensor, off, [list(t.ap[0])] + [list(d) for d in dims])


def build_nc(io_bufs: int = 6, mid_bufs: int = 4,
             load_split: int = 4) -> bass.Bass:
    # Bacc (not raw Bass): its compile() runs generate_event_semaphores,
    # which splits multi-sem waits to satisfy the TRN2 1-wait-per-
    # instruction ISA limit.
    nc = bacc.Bacc("TRN2", target_bir_lowering=False, debug=False,
                   num_devices=N_CORES)
    x = nc.dram_tensor("x", [ROWS, HW], F32, kind="ExternalInput")
    out = nc.dram_tensor("out", [ROWS, H_OUT * W_OUT], F32,
                         kind="ExternalOutput")

    with TileContext(nc) as tc:
        with tc.tile_pool(name="io", bufs=io_bufs) as io_pool, \
             tc.tile_pool(name="mid", bufs=mid_bufs) as mid_pool:
            for t in range(NTILES):
                row0 = t * CBLK
                xt = io_pool.tile([CBLK, HW], F32, name="xt", tag="xt")
                # Split each load across the SP (HWDGE) and Pool (SWDGE)
                # sequencers: a single issue path sustains only ~300 GB/s,
                # the halves transfer in parallel, and neither engine has
                # compute work that could stall its in-order stream.
                step = HW // load_split
                for p in range(load_split):
                    load_eng = nc.sync if p % 2 == 0 else nc.gpsimd
                    load_eng.dma_start(
                        out=xt[:, p * step:(p + 1) * step],
                        in_=x[row0:row0 + CBLK, p * step:(p + 1) * step])

                # Stage 1 (reduce W): yt layout [c, (j, h)], j-major.
                yt = mid_pool.tile([CBLK, W_OUT * H_IN], F32, name="yt",
                                   tag="yt")
                # 12-size bins j in {1,4}: [c, h(64), jpair(2,32), w(12)]
                src = _ap(xt, 10, [[W_IN, H_IN], [32, 2], [1, 12]])
                dst = _ap(yt, H_IN, [[1, H_IN], [192, 2]])
                nc.vector.reduce_sum(dst, src, axis=mybir.AxisListType.X)
                # 11-size bins j in {0,3}x{2,5}: starts {0,32}x{0,21}
                src = _ap(xt, 0, [[W_IN, H_IN], [32, 2], [21, 2], [1, 11]])
                dst = _ap(yt, 0, [[1, H_IN], [192, 2], [128, 2]])
                nc.vector.reduce_sum(dst, src, axis=mybir.AxisListType.X)

                # Stage 2 (reduce H): ot is [c, 36] i-major (col = i*6+j).
                ot = mid_pool.tile([CBLK, H_OUT * W_OUT], F32, name="ot",
                                   tag="ot")
                # 12-size bins i in {1,4}
                src = _ap(yt, 10, [[H_IN, W_OUT], [32, 2], [1, 12]])
                dst = _ap(ot, W_OUT, [[1, W_OUT], [18, 2]])
                nc.vector.reduce_sum(dst, src, axis=mybir.AxisListType.X)
                # 11-size bins i in {0,3}x{2,5}
                src = _ap(yt, 0, [[H_IN, W_OUT], [32, 2], [21, 2], [1, 11]])
                dst = _ap(ot, 0, [[1, W_OUT], [18, 2], [12, 2]])
                nc.vector.reduce_sum(dst, src, axis=mybir.AxisListType.X)

                # Mean scale 1/(h_size*w_size): 9 stride-regular 2x2 (i, j)
                # pair-groups, in-place on the otherwise-idle ACT engine so
                # the DVE does only the reduces.
                for (i0, _i1), hsz in _SGROUPS:
                    for (j0, _j1), wsz in _SGROUPS:
                        dst = _ap(ot, i0 * W_OUT + j0, [[18, 2], [3, 2]])
                        nc.scalar.mul(dst, dst, 1.0 / (hsz * wsz))

                # Store via the ACT sequencer (HWDGE): it directly follows
                # the scale muls in ACT program order, and keeps the SP
                # sequencer free to issue loads far ahead (a store waiting
                # on compute would otherwise stall SP's in-order stream).
                nc.scalar.dma_start(out=out[row0:row0 + CBLK, :], in_=ot)
    nc.compile()
    return nc


_NC_CACHE = None


def _get_nc() -> bass.Bass:
    global _NC_CACHE
    if _NC_CACHE is None:
        _NC_CACHE = build_nc()
    return _NC_CACHE


def run(x: np.ndarray, **spmd_kwargs):
    """Shard, run on 8 cores, gather.  Returns (output, BassKernelResults)."""
    x = np.ascontiguousarray(np.asarray(x), dtype=np.float32)
    assert x.shape == (N, C, H_IN, W_IN)
    in_maps = [
        {"x": x[i * N_PER_CORE:(i + 1) * N_PER_CORE].reshape(ROWS, HW)}
        for i in range(N_CORES)
    ]
    res = run_bass_kernel_spmd(_get_nc(), in_maps, list(range(N_CORES)),
                               **spmd_kwargs)
    out = np.concatenate(
        [res.results[i]["out"].reshape(N_PER_CORE, C, H_OUT, W_OUT)
         for i in range(N_CORES)], axis=0)
    return out, res


def kernel(x: np.ndarray) -> np.ndarray:
    out, _ = run(x)
    return out



# revision 14
# speedup vs baseline: 1.8144x; 1.8144x over previous
"""Adaptive average pool 2D (64x64 -> 6x6) on 8 TRN2 NeuronCores.

Input  x: (16, 2048, 64, 64) f32
Output  : (16, 2048, 6, 6) f32

Sharding: data-parallel over the batch dim (2 batches per core).
Per-core kernel: 32 tiles of [128 channels (partitions), 4096 (h*w)].

v3: 4-way engine balance (~2.75 us/tile each instead of DVE-bound 5.2):
  SP   loads cols [0, L_SP)                            ~2.72 us
  ACT  loads cols [L_SP, L_ACT) + batched stores       ~2.70 us
  Pool loads cols [L_ACT, 4096) + W-stage add-tree for
       h in [0, HP) + H-stage add-tree + scale mult    ~2.75 us
  DVE  W-stage strided reduces for h in [HP, 64)       ~2.74 us
gpsimd tensor_reduce only does partition-axis reductions, so the Pool
engine's share of the pooling runs as pairwise tensor_add trees
(~62 elems/row vs the DVE reduce's 68).  Pool's H-stage + scale for
tile t are emitted one iteration late so its in-order stream never
waits on the DVE's same-tile W-stage.
"""

import numpy as np

import concourse.bass as bass
import concourse.bacc as bacc
import concourse.mybir as mybir
from concourse.ap import AP
from concourse.tile import TileContext
from concourse.bass_utils import run_bass_kernel_spmd

# ---------------------------------------------------------------- problem dims
N, C, H_IN, W_IN = 16, 2048, 64, 64
H_OUT = W_OUT = 6
N_CORES = 8
N_PER_CORE = N // N_CORES          # 2
CBLK = 128
ROWS = N_PER_CORE * C              # 4096 (n,c) rows per core
NTILES = ROWS // CBLK              # 32
HW = H_IN * W_IN                   # 4096
F32 = mybir.dt.float32
ADD = mybir.AluOpType.add

# Adaptive pool bin edges (floor/ceil rule, same for H and W since 64->6):
# sizes [11,12,11,11,12,11] at starts [0,10,21,32,42,53].  After a pairwise
# (stride-2) first level, bins become sizes [5,6,5,5,6,5] at pair-starts
# [0,5,11,16,21,27], plus one raw single (x[10],x[21],x[42],x[53] -- the
# overlap elements) folded into each 5-bin.  The 6-bins {1,4} sit at
# stride 16 in pair space; the 5-bins {0,2,3,5} decompose {0,16}x{0,11}.

# Work split knobs (tuned against CoreSim).  DVE's pipeline fill is ~3 us
# (DMA init+sem latency before its first reduce), so the split equalizes
# engine END times, not busy times: DVE gets slightly less than the
# busy-balanced share.
L_SP = 1792        # SP loads cols [0, L_SP)
L_ACT = 3520       # ACT loads cols [L_SP, L_ACT); Pool loads the rest
HP = 28            # Pool W-tree covers h in [0, HP); DVE reduces [HP, 64)
# Output store groups (tiles per ACT store DMA); smaller final groups
# shorten the post-loop drain.
SGRPS = (8, 8, 8, 4, 4)
SGRP_START = tuple(sum(SGRPS[:i]) for i in range(len(SGRPS)))


def _ap(t: bass.AP, off: int, dims) -> AP:
    """Custom strided AP on a pool tile (keeps the tile's partition dim)."""
    assert t.offset == 0
    return AP(t.tensor, off, [list(t.ap[0])] + [list(d) for d in dims])


def _wtree(nc, eng, dst_yt, xt, h0, h1, pool):
    """W-stage 64->6 pooling for h in [h0, h1) as a pairwise add tree on
    `eng`.  Writes bin sums into yt's j-major layout (col = j*64 + h)."""
    nh = h1 - h0
    tp = pool.tile([CBLK, nh * 32], F32, name="tp", tag="tp")
    u6 = pool.tile([CBLK, nh * 6], F32, name="u6", tag="u6")
    v2 = pool.tile([CBLK, nh * 2], F32, name="v2", tag="v2")
    u5 = pool.tile([CBLK, nh * 8], F32, name="u5", tag="u5")
    w2 = pool.tile([CBLK, nh * 4], F32, name="w2", tag="w2")
    x0 = h0 * 64
    # L1: tp[c, h*32 + k] = x[2k] + x[2k+1]
    eng.tensor_tensor(_ap(tp, 0, [[32, nh], [1, 32]]),
                      _ap(xt, x0, [[64, nh], [2, 32]]),
                      _ap(xt, x0 + 1, [[64, nh], [2, 32]]), ADD)
    # 6-bins (j 1,4 at pair-starts 5,21): 6 -> 3 -> (1+2) -> 1
    eng.tensor_tensor(_ap(u6, 0, [[6, nh], [3, 2], [1, 3]]),
                      _ap(tp, 5, [[32, nh], [16, 2], [2, 3]]),
                      _ap(tp, 6, [[32, nh], [16, 2], [2, 3]]), ADD)
    eng.tensor_tensor(_ap(v2, 0, [[2, nh], [1, 2]]),
                      _ap(u6, 0, [[6, nh], [3, 2]]),
                      _ap(u6, 1, [[6, nh], [3, 2]]), ADD)
    eng.tensor_tensor(_ap(dst_yt, H_IN + h0, [[1, nh], [192, 2]]),
                      _ap(v2, 0, [[2, nh], [1, 2]]),
                      _ap(u6, 2, [[6, nh], [3, 2]]), ADD)
    # 5-bins (j 0,2,3,5 at pair-starts {0,16}x{0,11}): two pair rounds,
    # then the pair-singles {4,20}x{0,11}, then raw singles {10,42}x{0,11}
    eng.tensor_tensor(_ap(u5, 0, [[8, nh], [4, 2], [2, 2], [1, 2]]),
                      _ap(tp, 0, [[32, nh], [16, 2], [11, 2], [2, 2]]),
                      _ap(tp, 1, [[32, nh], [16, 2], [11, 2], [2, 2]]), ADD)
    eng.tensor_tensor(_ap(w2, 0, [[4, nh], [2, 2], [1, 2]]),
                      _ap(u5, 0, [[8, nh], [4, 2], [2, 2]]),
                      _ap(u5, 1, [[8, nh], [4, 2], [2, 2]]), ADD)
    eng.tensor_tensor(_ap(w2, 0, [[4, nh], [2, 2], [1, 2]]),
                      _ap(w2, 0, [[4, nh], [2, 2], [1, 2]]),
                      _ap(tp, 4, [[32, nh], [16, 2], [11, 2]]), ADD)
    eng.tensor_tensor(_ap(dst_yt, h0, [[1, nh], [192, 2], [128, 2]]),
                      _ap(w2, 0, [[4, nh], [2, 2], [1, 2]]),
                      _ap(xt, x0 + 10, [[64, nh], [32, 2], [11, 2]]), ADD)


def _htree(nc, eng, obig, ob, yt, pool):
    """H-stage 64->6 over yt's 6 j-columns as a Pool add tree, writing the
    i-major [c, 36] block at obig[:, ob:ob+36]."""
    th = pool.tile([CBLK, 6 * 32], F32, name="th", tag="th")
    g6 = pool.tile([CBLK, 6 * 6], F32, name="g6", tag="g6")
    gv = pool.tile([CBLK, 6 * 2], F32, name="gv", tag="gv")
    g5 = pool.tile([CBLK, 6 * 8], F32, name="g5", tag="g5")
    gw = pool.tile([CBLK, 6 * 4], F32, name="gw", tag="gw")
    eng.tensor_tensor(_ap(th, 0, [[32, 6], [1, 32]]),
                      _ap(yt, 0, [[64, 6], [2, 32]]),
                      _ap(yt, 1, [[64, 6], [2, 32]]), ADD)
    # 6-bins (i 1,4)
    eng.tensor_tensor(_ap(g6, 0, [[6, 6], [3, 2], [1, 3]]),
                      _ap(th, 5, [[32, 6], [16, 2], [2, 3]]),
                      _ap(th, 6, [[32, 6], [16, 2], [2, 3]]), ADD)
    eng.tensor_tensor(_ap(gv, 0, [[2, 6], [1, 2]]),
                      _ap(g6, 0, [[6, 6], [3, 2]]),
                      _ap(g6, 1, [[6, 6], [3, 2]]), ADD)
    eng.tensor_tensor(_ap(obig, ob + 6, [[1, 6], [18, 2]]),
                      _ap(gv, 0, [[2, 6], [1, 2]]),
                      _ap(g6, 2, [[6, 6], [3, 2]]), ADD)
    # 5-bins (i 0,2,3,5)
    eng.tensor_tensor(_ap(g5, 0, [[8, 6], [4, 2], [2, 2], [1, 2]]),
                      _ap(th, 0, [[32, 6], [16, 2], [11, 2], [2, 2]]),
                      _ap(th, 1, [[32, 6], [16, 2], [11, 2], [2, 2]]), ADD)
    eng.tensor_tensor(_ap(gw, 0, [[4, 6], [2, 2], [1, 2]]),
                      _ap(g5, 0, [[8, 6], [4, 2], [2, 2]]),
                      _ap(g5, 1, [[8, 6], [4, 2], [2, 2]]), ADD)
    eng.tensor_tensor(_ap(gw, 0, [[4, 6], [2, 2], [1, 2]]),
                      _ap(gw, 0, [[4, 6], [2, 2], [1, 2]]),
                      _ap(th, 4, [[32, 6], [16, 2], [11, 2]]), ADD)
    eng.tensor_tensor(_ap(obig, ob, [[1, 6], [18, 2], [12, 2]]),
                      _ap(gw, 0, [[4, 6], [2, 2], [1, 2]]),
                      _ap(yt, 10, [[64, 6], [32, 2], [11, 2]]), ADD)


def build_nc(io_bufs: int = 7, mid_bufs: int = 3) -> bass.Bass:
    # Bacc (not raw Bass): its compile() runs generate_event_semaphores,
    # which splits multi-sem waits to satisfy the TRN2 1-wait-per-
    # instruction ISA limit.
    nc = bacc.Bacc("TRN2", target_bir_lowering=False, debug=False,
                   num_devices=N_CORES)
    x = nc.dram_tensor("x", [ROWS, HW], F32, kind="ExternalInput")
    out = nc.dram_tensor("out", [ROWS, H_OUT * W_OUT], F32,
                         kind="ExternalOutput")
    HD = 64 - HP                   # DVE h-range size

    with TileContext(nc) as tc:
        with tc.tile_pool(name="cst", bufs=1) as cst_pool, \
             tc.tile_pool(name="io", bufs=io_bufs) as io_pool, \
             tc.tile_pool(name="mid", bufs=mid_bufs) as mid_pool, \
             tc.tile_pool(name="og", bufs=2) as out_pool:
            # [128, 36] tile of 1/(h_size*w_size), bin col = i*6+j.  Nine
            # memsets, one per (h-class, w-class) pair group.
            coeff = cst_pool.tile([CBLK, H_OUT * W_OUT], F32, name="coeff")
            sgrp = [(1, 12), (0, 11), (2, 11)]   # (first bin of {b,b+3}, size)
            for i0, hsz in sgrp:
                for j0, wsz in sgrp:
                    dst = _ap(coeff, i0 * W_OUT + j0, [[18, 2], [3, 2]])
                    nc.gpsimd.memset(dst, 1.0 / (hsz * wsz))

            def dve_s1(yt, xt, h0, h1):
                """DVE W-stage strided reduces for h in [h0, h1)."""
                nh = h1 - h0
                nc.vector.reduce_sum(
                    _ap(yt, H_IN + h0, [[1, nh], [192, 2]]),
                    _ap(xt, h0 * 64 + 10, [[64, nh], [32, 2], [1, 12]]),
                    axis=mybir.AxisListType.X)
                nc.vector.reduce_sum(
                    _ap(yt, h0, [[1, nh], [192, 2], [128, 2]]),
                    _ap(xt, h0 * 64, [[64, nh], [32, 2], [21, 2], [1, 11]]),
                    axis=mybir.AxisListType.X)

            obig = prev_yt = yt = None
            grp = glen = None
            for t in range(NTILES + 1):
                if t < NTILES:
                    row0 = t * CBLK
                    xrow = x[row0:row0 + CBLK, :]
                    xt = io_pool.tile([CBLK, HW], F32, name="xt", tag="xt")
                    if t == 0:
                        # Pipeline-fill surgery: small first pieces so the
                        # compute engines' first sems land ~3.1 us instead
                        # of ~5; W-tree/s1 are split to start on partials.
                        for c0, c1 in ((0, 256), (256, 1024), (1024, 1792)):
                            nc.sync.dma_start(out=xt[:, c0:c1],
                                              in_=xrow[:, c0:c1])
                        for c0, c1 in ((1792, 2048), (2048, 2752),
                                       (2752, 3520)):
                            nc.scalar.dma_start(out=xt[:, c0:c1],
                                                in_=xrow[:, c0:c1])
                        nc.gpsimd.dma_start(out=xt[:, 3520:],
                                            in_=xrow[:, 3520:])
                    else:
                        nc.sync.dma_start(out=xt[:, :L_SP],
                                          in_=xrow[:, :L_SP])
                        nc.scalar.dma_start(out=xt[:, L_SP:L_ACT],
                                            in_=xrow[:, L_SP:L_ACT])
                        nc.gpsimd.dma_start(out=xt[:, L_ACT:],
                                            in_=xrow[:, L_ACT:])

                # Pool H-stage + scale for the PREVIOUS tile (one-iteration
                # lag keeps Pool's in-order stream from waiting on the DVE).
                if t >= 1:
                    tm = t - 1
                    if tm in SGRP_START:
                        gi = SGRP_START.index(tm)
                        grp, glen = tm, SGRPS[gi]
                        obig = out_pool.tile([CBLK, glen * 36], F32,
                                             name="obig", tag="obig")
                    ob = (tm - grp) * 36
                    _htree(nc, nc.gpsimd, obig, ob, prev_yt, mid_pool)
                    nc.gpsimd.tensor_tensor(_ap(obig, ob, [[1, 36]]),
                                            _ap(obig, ob, [[1, 36]]),
                                            _ap(coeff, 0, [[1, 36]]),
                                            mybir.AluOpType.mult)
                    if tm == grp + glen - 1:
                        base = out[0:CBLK, :]
                        dst = AP(base.tensor, grp * CBLK * 36,
                                 [[36, CBLK], [CBLK * 36, glen], [1, 36]])
                        nc.scalar.dma_start(out=dst, in_=obig[:, :])

                if t < NTILES:
                    # yt: stage-1 result, [c, (j, h)] j-major (col = j*64+h)
                    yt = mid_pool.tile([CBLK, W_OUT * H_IN], F32, name="yt",
                                       tag="yt")
                    if t == 0:
                        # Emission order = data-arrival order of the pieces.
                        dve_s1(yt, xt, 28, 32)     # after ACT piece 1
                        dve_s1(yt, xt, 55, 64)     # after Pool load
                        _wtree(nc, nc.gpsimd, yt, xt, 0, 16, mid_pool)
                        dve_s1(yt, xt, 32, 43)     # after ACT piece 2
                        _wtree(nc, nc.gpsimd, yt, xt, 16, 28, mid_pool)
                        dve_s1(yt, xt, 43, 55)     # after ACT piece 3
                    else:
                        _wtree(nc, nc.gpsimd, yt, xt, 0, HP, mid_pool)
                        dve_s1(yt, xt, HP, 64)
                    prev_yt = yt
    nc.compile()
    return nc


_NC_CACHE = None


def _get_nc() -> bass.Bass:
    global _NC_CACHE
    if _NC_CACHE is None:
        _NC_CACHE = build_nc()
    return _NC_CACHE


def run(x: np.ndarray, **spmd_kwargs):
    """Shard, run on 8 cores, gather.  Returns (output, BassKernelResults)."""
    x = np.ascontiguousarray(np.asarray(x), dtype=np.float32)
    assert x.shape == (N, C, H_IN, W_IN)
    in_maps = [
        {"x": x[i * N_PER_CORE:(i + 1) * N_PER_CORE].reshape(ROWS, HW)}
        for i in range(N_CORES)
    ]
    res = run_bass_kernel_spmd(_get_nc(), in_maps, list(range(N_CORES)),
                               **spmd_kwargs)
    out = np.concatenate(
        [res.results[i]["out"].reshape(N_PER_CORE, C, H_OUT, W_OUT)
         for i in range(N_CORES)], axis=0)
    return out, res


def kernel(x: np.ndarray) -> np.ndarray:
    out, _ = run(x)
    return out


# revision 15
# speedup vs baseline: 1.8262x; 1.0065x over previous
"""Adaptive average pool 2D (64x64 -> 6x6) on 8 TRN2 NeuronCores.

Input  x: (16, 2048, 64, 64) f32
Output  : (16, 2048, 6, 6) f32

Sharding: data-parallel over the batch dim (2 batches per core).
Per-core kernel: 32 tiles of [128 channels (partitions), 4096 (h*w)].

v3: 4-way engine balance (~2.75 us/tile each instead of DVE-bound 5.2):
  SP   loads cols [0, L_SP)                            ~2.72 us
  ACT  loads cols [L_SP, L_ACT) + batched stores       ~2.70 us
  Pool loads cols [L_ACT, 4096) + W-stage add-tree for
       h in [0, HP) + H-stage add-tree + scale mult    ~2.75 us
  DVE  W-stage strided reduces for h in [HP, 64)       ~2.74 us
gpsimd tensor_reduce only does partition-axis reductions, so the Pool
engine's share of the pooling runs as pairwise tensor_add trees
(~62 elems/row vs the DVE reduce's 68).  Pool's H-stage + scale for
tile t are emitted one iteration late so its in-order stream never
waits on the DVE's same-tile W-stage.
"""

import numpy as np

import concourse.bass as bass
import concourse.bacc as bacc
import concourse.mybir as mybir
from concourse.ap import AP
from concourse.tile import TileContext
from concourse.bass_utils import run_bass_kernel_spmd

# ---------------------------------------------------------------- problem dims
N, C, H_IN, W_IN = 16, 2048, 64, 64
H_OUT = W_OUT = 6
N_CORES = 8
N_PER_CORE = N // N_CORES          # 2
CBLK = 128
ROWS = N_PER_CORE * C              # 4096 (n,c) rows per core
NTILES = ROWS // CBLK              # 32
HW = H_IN * W_IN                   # 4096
F32 = mybir.dt.float32
ADD = mybir.AluOpType.add

# Adaptive pool bin edges (floor/ceil rule, same for H and W since 64->6):
# sizes [11,12,11,11,12,11] at starts [0,10,21,32,42,53].  After a pairwise
# (stride-2) first level, bins become sizes [5,6,5,5,6,5] at pair-starts
# [0,5,11,16,21,27], plus one raw single (x[10],x[21],x[42],x[53] -- the
# overlap elements) folded into each 5-bin.  The 6-bins {1,4} sit at
# stride 16 in pair space; the 5-bins {0,2,3,5} decompose {0,16}x{0,11}.

# Work split knobs (tuned against CoreSim).  DVE's pipeline fill is ~3 us
# (DMA init+sem latency before its first reduce), so the split equalizes
# engine END times, not busy times: DVE gets slightly less than the
# busy-balanced share.
L_SP = 1776        # SP loads cols [0, L_SP)
L_ACT = 3490       # ACT loads cols [L_SP, L_ACT); Pool loads the rest
HP = 28            # Pool W-tree covers h in [0, HP); DVE reduces [HP, 64)
# Output store groups (tiles per ACT store DMA); smaller final groups
# shorten the post-loop drain.
SGRPS = (8, 8, 8, 4, 4)
SGRP_START = tuple(sum(SGRPS[:i]) for i in range(len(SGRPS)))


def _ap(t: bass.AP, off: int, dims) -> AP:
    """Custom strided AP on a pool tile (keeps the tile's partition dim)."""
    assert t.offset == 0
    return AP(t.tensor, off, [list(t.ap[0])] + [list(d) for d in dims])


def _wtree(nc, eng, dst_yt, xt, h0, h1, pool):
    """W-stage 64->6 pooling for h in [h0, h1) as a pairwise add tree on
    `eng`.  Writes bin sums into yt's j-major layout (col = j*64 + h)."""
    nh = h1 - h0
    tp = pool.tile([CBLK, nh * 32], F32, name="tp", tag="tp")
    u6 = pool.tile([CBLK, nh * 6], F32, name="u6", tag="u6")
    v2 = pool.tile([CBLK, nh * 2], F32, name="v2", tag="v2")
    u5 = pool.tile([CBLK, nh * 8], F32, name="u5", tag="u5")
    w2 = pool.tile([CBLK, nh * 4], F32, name="w2", tag="w2")
    x0 = h0 * 64
    # L1: tp[c, h*32 + k] = x[2k] + x[2k+1]
    eng.tensor_tensor(_ap(tp, 0, [[32, nh], [1, 32]]),
                      _ap(xt, x0, [[64, nh], [2, 32]]),
                      _ap(xt, x0 + 1, [[64, nh], [2, 32]]), ADD)
    # 6-bins (j 1,4 at pair-starts 5,21): 6 -> 3 -> (1+2) -> 1
    eng.tensor_tensor(_ap(u6, 0, [[6, nh], [3, 2], [1, 3]]),
                      _ap(tp, 5, [[32, nh], [16, 2], [2, 3]]),
                      _ap(tp, 6, [[32, nh], [16, 2], [2, 3]]), ADD)
    eng.tensor_tensor(_ap(v2, 0, [[2, nh], [1, 2]]),
                      _ap(u6, 0, [[6, nh], [3, 2]]),
                      _ap(u6, 1, [[6, nh], [3, 2]]), ADD)
    eng.tensor_tensor(_ap(dst_yt, H_IN + h0, [[1, nh], [192, 2]]),
                      _ap(v2, 0, [[2, nh], [1, 2]]),
                      _ap(u6, 2, [[6, nh], [3, 2]]), ADD)
    # 5-bins (j 0,2,3,5 at pair-starts {0,16}x{0,11}): two pair rounds,
    # then the pair-singles {4,20}x{0,11}, then raw singles {10,42}x{0,11}
    eng.tensor_tensor(_ap(u5, 0, [[8, nh], [4, 2], [2, 2], [1, 2]]),
                      _ap(tp, 0, [[32, nh], [16, 2], [11, 2], [2, 2]]),
                      _ap(tp, 1, [[32, nh], [16, 2], [11, 2], [2, 2]]), ADD)
    eng.tensor_tensor(_ap(w2, 0, [[4, nh], [2, 2], [1, 2]]),
                      _ap(u5, 0, [[8, nh], [4, 2], [2, 2]]),
                      _ap(u5, 1, [[8, nh], [4, 2], [2, 2]]), ADD)
    eng.tensor_tensor(_ap(w2, 0, [[4, nh], [2, 2], [1, 2]]),
                      _ap(w2, 0, [[4, nh], [2, 2], [1, 2]]),
                      _ap(tp, 4, [[32, nh], [16, 2], [11, 2]]), ADD)
    eng.tensor_tensor(_ap(dst_yt, h0, [[1, nh], [192, 2], [128, 2]]),
                      _ap(w2, 0, [[4, nh], [2, 2], [1, 2]]),
                      _ap(xt, x0 + 10, [[64, nh], [32, 2], [11, 2]]), ADD)


def _htree(nc, eng, obig, ob, yt, pool):
    """H-stage 64->6 over yt's 6 j-columns as a Pool add tree, writing the
    i-major [c, 36] block at obig[:, ob:ob+36]."""
    th = pool.tile([CBLK, 6 * 32], F32, name="th", tag="th")
    g6 = pool.tile([CBLK, 6 * 6], F32, name="g6", tag="g6")
    gv = pool.tile([CBLK, 6 * 2], F32, name="gv", tag="gv")
    g5 = pool.tile([CBLK, 6 * 8], F32, name="g5", tag="g5")
    gw = pool.tile([CBLK, 6 * 4], F32, name="gw", tag="gw")
    eng.tensor_tensor(_ap(th, 0, [[32, 6], [1, 32]]),
                      _ap(yt, 0, [[64, 6], [2, 32]]),
                      _ap(yt, 1, [[64, 6], [2, 32]]), ADD)
    # 6-bins (i 1,4)
    eng.tensor_tensor(_ap(g6, 0, [[6, 6], [3, 2], [1, 3]]),
                      _ap(th, 5, [[32, 6], [16, 2], [2, 3]]),
                      _ap(th, 6, [[32, 6], [16, 2], [2, 3]]), ADD)
    eng.tensor_tensor(_ap(gv, 0, [[2, 6], [1, 2]]),
                      _ap(g6, 0, [[6, 6], [3, 2]]),
                      _ap(g6, 1, [[6, 6], [3, 2]]), ADD)
    eng.tensor_tensor(_ap(obig, ob + 6, [[1, 6], [18, 2]]),
                      _ap(gv, 0, [[2, 6], [1, 2]]),
                      _ap(g6, 2, [[6, 6], [3, 2]]), ADD)
    # 5-bins (i 0,2,3,5)
    eng.tensor_tensor(_ap(g5, 0, [[8, 6], [4, 2], [2, 2], [1, 2]]),
                      _ap(th, 0, [[32, 6], [16, 2], [11, 2], [2, 2]]),
                      _ap(th, 1, [[32, 6], [16, 2], [11, 2], [2, 2]]), ADD)
    eng.tensor_tensor(_ap(gw, 0, [[4, 6], [2, 2], [1, 2]]),
                      _ap(g5, 0, [[8, 6], [4, 2], [2, 2]]),
                      _ap(g5, 1, [[8, 6], [4, 2], [2, 2]]), ADD)
    eng.tensor_tensor(_ap(gw, 0, [[4, 6], [2, 2], [1, 2]]),
                      _ap(gw, 0, [[4, 6], [2, 2], [1, 2]]),
                      _ap(th, 4, [[32, 6], [16, 2], [11, 2]]), ADD)
    eng.tensor_tensor(_ap(obig, ob, [[1, 6], [18, 2], [12, 2]]),
                      _ap(gw, 0, [[4, 6], [2, 2], [1, 2]]),
                      _ap(yt, 10, [[64, 6], [32, 2], [11, 2]]), ADD)


def build_nc(io_bufs: int = 7, mid_bufs: int = 3) -> bass.Bass:
    # Bacc (not raw Bass): its compile() runs generate_event_semaphores,
    # which splits multi-sem waits to satisfy the TRN2 1-wait-per-
    # instruction ISA limit.
    nc = bacc.Bacc("TRN2", target_bir_lowering=False, debug=False,
                   num_devices=N_CORES)
    x = nc.dram_tensor("x", [ROWS, HW], F32, kind="ExternalInput")
    out = nc.dram_tensor("out", [ROWS, H_OUT * W_OUT], F32,
                         kind="ExternalOutput")
    HD = 64 - HP                   # DVE h-range size

    with TileContext(nc) as tc:
        with tc.tile_pool(name="cst", bufs=1) as cst_pool, \
             tc.tile_pool(name="io", bufs=io_bufs) as io_pool, \
             tc.tile_pool(name="mid", bufs=mid_bufs) as mid_pool, \
             tc.tile_pool(name="og", bufs=2) as out_pool:
            # [128, 36] tile of 1/(h_size*w_size), bin col = i*6+j.  Nine
            # memsets, one per (h-class, w-class) pair group.
            coeff = cst_pool.tile([CBLK, H_OUT * W_OUT], F32, name="coeff")
            sgrp = [(1, 12), (0, 11), (2, 11)]   # (first bin of {b,b+3}, size)
            for i0, hsz in sgrp:
                for j0, wsz in sgrp:
                    dst = _ap(coeff, i0 * W_OUT + j0, [[18, 2], [3, 2]])
                    nc.gpsimd.memset(dst, 1.0 / (hsz * wsz))

            def dve_s1(yt, xt, h0, h1):
                """DVE W-stage strided reduces for h in [h0, h1)."""
                nh = h1 - h0
                nc.vector.reduce_sum(
                    _ap(yt, H_IN + h0, [[1, nh], [192, 2]]),
                    _ap(xt, h0 * 64 + 10, [[64, nh], [32, 2], [1, 12]]),
                    axis=mybir.AxisListType.X)
                nc.vector.reduce_sum(
                    _ap(yt, h0, [[1, nh], [192, 2], [128, 2]]),
                    _ap(xt, h0 * 64, [[64, nh], [32, 2], [21, 2], [1, 11]]),
                    axis=mybir.AxisListType.X)

            obig = prev_yt = yt = None
            grp = glen = None
            for t in range(NTILES + 1):
                if t < NTILES:
                    row0 = t * CBLK
                    xrow = x[row0:row0 + CBLK, :]
                    xt = io_pool.tile([CBLK, HW], F32, name="xt", tag="xt")
                    if t == 0:
                        # Pipeline-fill surgery: small first pieces so the
                        # compute engines' first sems land ~3.1 us instead
                        # of ~5; W-tree/s1 are split to start on partials.
                        for c0, c1 in ((0, 256), (256, 1024), (1024, 1792)):
                            nc.sync.dma_start(out=xt[:, c0:c1],
                                              in_=xrow[:, c0:c1])
                        for c0, c1 in ((1792, 2048), (2048, 2752),
                                       (2752, 3520)):
                            nc.scalar.dma_start(out=xt[:, c0:c1],
                                                in_=xrow[:, c0:c1])
                        nc.gpsimd.dma_start(out=xt[:, 3520:],
                                            in_=xrow[:, 3520:])
                    else:
                        nc.sync.dma_start(out=xt[:, :L_SP],
                                          in_=xrow[:, :L_SP])
                        nc.scalar.dma_start(out=xt[:, L_SP:L_ACT],
                                            in_=xrow[:, L_SP:L_ACT])
                        nc.gpsimd.dma_start(out=xt[:, L_ACT:],
                                            in_=xrow[:, L_ACT:])

                # Pool H-stage + scale for the PREVIOUS tile (one-iteration
                # lag keeps Pool's in-order stream from waiting on the DVE).
                if t >= 1:
                    tm = t - 1
                    if tm in SGRP_START:
                        gi = SGRP_START.index(tm)
                        grp, glen = tm, SGRPS[gi]
                        obig = out_pool.tile([CBLK, glen * 36], F32,
                                             name="obig", tag="obig")
                    ob = (tm - grp) * 36
                    _htree(nc, nc.gpsimd, obig, ob, prev_yt, mid_pool)
                    nc.gpsimd.tensor_tensor(_ap(obig, ob, [[1, 36]]),
                                            _ap(obig, ob, [[1, 36]]),
                                            _ap(coeff, 0, [[1, 36]]),
                                            mybir.AluOpType.mult)
                    if tm == grp + glen - 1:
                        base = out[0:CBLK, :]
                        dst = AP(base.tensor, grp * CBLK * 36,
                                 [[36, CBLK], [CBLK * 36, glen], [1, 36]])
                        nc.scalar.dma_start(out=dst, in_=obig[:, :])

                if t < NTILES:
                    # yt: stage-1 result, [c, (j, h)] j-major (col = j*64+h)
                    yt = mid_pool.tile([CBLK, W_OUT * H_IN], F32, name="yt",
                                       tag="yt")
                    if t == 0:
                        # Emission order = data-arrival order of the pieces.
                        dve_s1(yt, xt, 28, 32)     # after ACT piece 1
                        dve_s1(yt, xt, 55, 64)     # after Pool load
                        _wtree(nc, nc.gpsimd, yt, xt, 0, 16, mid_pool)
                        dve_s1(yt, xt, 32, 43)     # after ACT piece 2
                        _wtree(nc, nc.gpsimd, yt, xt, 16, 28, mid_pool)
                        dve_s1(yt, xt, 43, 55)     # after ACT piece 3
                    else:
                        _wtree(nc, nc.gpsimd, yt, xt, 0, HP, mid_pool)
                        dve_s1(yt, xt, HP, 64)
                    prev_yt = yt
    nc.compile()
    return nc


_NC_CACHE = None


def _get_nc() -> bass.Bass:
    global _NC_CACHE
    if _NC_CACHE is None:
        _NC_CACHE = build_nc()
    return _NC_CACHE


def run(x: np.ndarray, **spmd_kwargs):
    """Shard, run on 8 cores, gather.  Returns (output, BassKernelResults)."""
    x = np.ascontiguousarray(np.asarray(x), dtype=np.float32)
    assert x.shape == (N, C, H_IN, W_IN)
    in_maps = [
        {"x": x[i * N_PER_CORE:(i + 1) * N_PER_CORE].reshape(ROWS, HW)}
        for i in range(N_CORES)
    ]
    res = run_bass_kernel_spmd(_get_nc(), in_maps, list(range(N_CORES)),
                               **spmd_kwargs)
    out = np.concatenate(
        [res.results[i]["out"].reshape(N_PER_CORE, C, H_OUT, W_OUT)
         for i in range(N_CORES)], axis=0)
    return out, res


def kernel(x: np.ndarray) -> np.ndarray:
    out, _ = run(x)
    return out
